# revision 1
# baseline (speedup 1.0000x reference)
"""Coattention kernel for Trainium2, data-parallel over batch across 8 NeuronCores.

Per core (one batch element b):
    Qp   = Q_b @ Wq                                  (bq == 0 in this problem)
    S0   = (C_b * w3) @ Qp^T          [c, q]
    u_i  = w1 . C_i    (per-row const -> cancels in row softmax)
    v_j  = w2 . Qp_j   (per-col const -> cancels in col softmax)
    E1   = exp(S0 + u) in [c, q] layout  (feeds col-softmax path)
    E2   = exp(S0^T + v) in [q, c] layout (feeds row-softmax path)
    T    = diag(1/colsum(E1)) @ E1^T @ C_b           == S_col^T @ C_b
    A    = diag(1/r) @ E2^T @ Qp,  r = E2^T @ 1      == S_row @ Qp
    Bm   = diag(1/r) @ E2^T @ T                      == S_row @ T
    out  = [C_b | A | C_b*A | C_b*Bm]                [c, 4d]

E is needed with both of its axes on partitions (contraction over c for T,
over q for A/Bm), so S0 is computed twice on the PE (both orientations) —
cheaper than transposing the 4MB exp(S) matrix.  The row/col constant terms
u, v are folded in as per-partition ACT biases; masks are all-ones and
b == bq == 0 per the problem spec, so they drop out.  Matmuls run in
float32r (~13-bit mantissa, 4x faster than fp32 on the PE).
"""

import os
import sys

import numpy as np

for _p in ("/opt/trn_rl_repo", "/root/.axon_site/_ro/trn_rl_repo"):
    if os.path.isdir(_p) and _p not in sys.path:
        sys.path.append(_p)

import concourse.bass as bass
import concourse.mybir as mybir
import concourse.tile as tile
from concourse.bass_utils import run_bass_kernel_spmd

C_LEN, Q_LEN, DIM, B = 2048, 512, 512, 8
N_CORES = 8
IC = C_LEN // 128   # 16 i-chunks
JC = Q_LEN // 128   # 4 j-chunks
KT = DIM // 128     # 4 k-tiles

F32 = mybir.dt.float32
F32R = mybir.dt.float32r
EXP = mybir.ActivationFunctionType.Exp
MULT = mybir.AluOpType.mult
ADD = mybir.AluOpType.add


def _split_multi_waits(nc, cap=1):
    """Walrus in this container rejects >1 sync wait per CTRL instruction;
    Tile's tail drain carries one wait per tracked processor.  Spill the
    extras onto preceding single-wait NoOps on the same engine."""
    for fn in nc.m.functions:
        for blk in fn.blocks:
            insts = list(blk.instructions)
            out, changed = [], False
            for inst in insts:
                si = inst.sync_info
                ow = si.on_wait if si is not None else None
                if ow is not None and len(ow) > cap:
                    waits = list(ow)
                    for w in waits[:-cap]:
                        nop = mybir.InstNoOp(
                            name=nc.get_next_instruction_name(), ins=[], outs=[]
                        )
                        nop.engine = inst.engine
                        nop.sync_info = mybir.SyncInfo(on_wait=[w], on_update=[])
                        out.append(nop)
                    si.on_wait = waits[-cap:]
                    changed = True
                out.append(inst)
            if changed:
                blk.instructions = out


def _build_program(split_waits=True, debug_taps=False):
    nc = bass.Bass()

    CT = nc.dram_tensor("CT", [DIM, C_LEN], F32, kind="ExternalInput")
    CN = nc.dram_tensor("CN", [C_LEN, DIM], F32, kind="ExternalInput")
    QT = nc.dram_tensor("QT", [DIM, Q_LEN], F32, kind="ExternalInput")
    WQ = nc.dram_tensor("WQ", [DIM, DIM], F32, kind="ExternalInput")
    W1B = nc.dram_tensor("W1B", [128, DIM], F32, kind="ExternalInput")
    W2B = nc.dram_tensor("W2B", [128, DIM], F32, kind="ExternalInput")
    W3C = nc.dram_tensor("W3C", [128, KT], F32, kind="ExternalInput")
    ONESC = nc.dram_tensor("ONESC", [128, 2], F32, kind="ExternalInput")
    Y = nc.dram_tensor("Y", [C_LEN, 4 * DIM], F32, kind="ExternalOutput")
    if debug_taps:
        T_TAP = nc.dram_tensor("T_TAP", [128, JC * DIM], F32, kind="ExternalOutput")
        B_TAP = nc.dram_tensor("B_TAP", [128, DIM], F32, kind="ExternalOutput")
        R_TAP = nc.dram_tensor("R_TAP", [128, 1], F32, kind="ExternalOutput")

    r = F32R

    with tile.TileContext(nc) as tc:
        with (
            tc.tile_pool(name="consts", bufs=1) as consts,
            tc.tile_pool(name="big", bufs=1) as big,
            tc.tile_pool(name="qtwq", bufs=1) as qtwq,
            tc.tile_pool(name="ps_mm", bufs=6, space="PSUM") as ps_mm,
            tc.tile_pool(name="ps_vec", bufs=2, space="PSUM") as ps_vec,
            tc.tile_pool(name="scr", bufs=2) as scr,
            tc.tile_pool(name="stage", bufs=4) as stagep,
        ):
            # ---- inputs (qt/wq first, chunked: P1's first MM needs kt=0 only) ----
            qt_sb = qtwq.tile([128, KT, Q_LEN], F32R)
            wq_sb = qtwq.tile([128, KT, DIM], F32R)
            for kt in range(KT):
                nc.sync.dma_start(
                    out=wq_sb[:, kt, :],
                    in_=WQ[kt * 128 : (kt + 1) * 128, :].bitcast(F32R),
                )
                # qt chunks on the gpsimd (SWDGE) queue: lands in parallel
                # with the wq chunks on the sync queue, so P1's first matmul
                # starts one chunk-transfer earlier.
                nc.gpsimd.dma_start(
                    out=qt_sb[:, kt, :],
                    in_=QT[kt * 128 : (kt + 1) * 128, :].bitcast(F32R),
                )

            # ---- constants / small tensors ----
            w1b = consts.tile([128, DIM], F32R)
            w2b = consts.tile([128, DIM], F32)
            w3c = consts.tile([128, KT], F32)
            ones_r = consts.tile([128, 2], F32R)
            nc.sync.dma_start(out=w3c, in_=W3C[:])
            nc.sync.dma_start(out=ones_r, in_=ONESC[:].bitcast(F32R))
            nc.sync.dma_start(out=w2b, in_=W2B[:])
            nc.sync.dma_start(out=w1b, in_=W1B[:].bitcast(F32R))

            u_sb = consts.tile([128, IC], F32)
            v_sb = consts.tile([128, JC], F32)
            rcs_sb = consts.tile([128, JC], F32)

            ct_sb = big.tile([128, KT, C_LEN], F32R)
            cn_sb = big.tile([128, IC, DIM], F32R)

            def _ct_dma(kt):
                nc.sync.dma_start(
                    out=ct_sb[:, kt, :],
                    in_=CT[kt * 128 : (kt + 1) * 128, :].bitcast(F32R),
                )

            def _cn_dma(g):
                nc.sync.dma_start(
                    out=cn_sb[:, g * 4 : (g + 1) * 4, :],
                    in_=CN[g * 512 : (g + 1) * 512, :]
                    .rearrange("(ic p) e -> p ic e", p=128)
                    .bitcast(F32R),
                )

            # ct gates the S0 matmuls, cn group 0 gates the first u
            # reduction (E1's exp bias): interleave so neither starves.
            _ct_dma(0); _ct_dma(1); _cn_dma(0); _ct_dma(2); _ct_dma(3)
            _cn_dma(1); _cn_dma(2); _cn_dma(3)
            # C passthrough output DMAs only need cn: issue them now so they
            # drain during the otherwise-idle mid-kernel DMA window.
            for ic in range(IC):
                nc.sync.dma_start(
                    out=Y[ic * 128 : (ic + 1) * 128, 0:DIM],
                    in_=cn_sb[:, ic, :].bitcast(F32),
                )

            qpw3t_sb = big.tile([128, KT * Q_LEN], F32R)  # w3-scaled Qp^T, kt-major
            qp_sb = big.tile([128, JC, DIM], F32R)        # Qp natural, jc-major
            e1_sb = big.tile([128, IC, Q_LEN], F32R)      # exp(S0 + u), [c, q]
            e2_sb = big.tile([128, JC, C_LEN], F32R)      # exp(S0^T + v), [q, c]
            t_sb = big.tile([128, JC * DIM], F32R)        # T = S_col^T @ C, jc-major

            # ---- P1: Qp^T (per e-chunk), scaled by w3 ----
            for m in range(KT):
                ps = ps_mm.tile([128, Q_LEN], F32, tag="mm")
                for kt in range(KT):
                    nc.tensor.matmul(
                        ps,
                        wq_sb[:, kt, m * 128 : (m + 1) * 128],
                        qt_sb[:, kt, :],
                        start=(kt == 0),
                        stop=(kt == KT - 1),
                    )
                nc.vector.tensor_scalar_mul(
                    qpw3t_sb[:, m * Q_LEN : (m + 1) * Q_LEN], ps, w3c[:, m : m + 1]
                )

            # ---- P2: Qp natural (per j-chunk) + v ----
            for jc in range(JC):
                ps = ps_mm.tile([128, DIM], F32, tag="mm")
                for kt in range(KT):
                    nc.tensor.matmul(
                        ps,
                        qt_sb[:, kt, jc * 128 : (jc + 1) * 128],
                        wq_sb[:, kt, :],
                        start=(kt == 0),
                        stop=(kt == KT - 1),
                    )
                nc.scalar.copy(qp_sb[:, jc, :], ps)
                sc = scr.tile([128, DIM], F32, tag="ttr")
                nc.vector.tensor_mul(sc, ps, w2b)
                nc.vector.reduce_sum(
                    v_sb[:, jc : jc + 1], sc, axis=mybir.AxisListType.X
                )

            # u reductions (DVE): after P1/P2's DVE work in the in-order
            # stream, but ahead of the E1 exps that consume u.
            for ic in range(IC):
                sc = scr.tile([128, DIM], F32, tag="ttr")
                nc.vector.tensor_mul(sc, cn_sb[:, ic, :], w1b)
                nc.vector.reduce_sum(
                    u_sb[:, ic : ic + 1], sc, axis=mybir.AxisListType.X
                )

            # ---- P4: S0 natural + E1.  First 8 chunks run kt-outer in two
            # 4-wide blocks so the S0 matmuls can start before the last ct
            # k-tiles have landed; the rest run ic-outer. ----
            for blk in range(2):
                pss = []
                for _i in range(4):
                    ps_blk = ps_mm.tile([128, Q_LEN], F32, tag="mm")
                    pss.append(ps_blk)
                for kt in range(KT):
                    for i4 in range(4):
                        ic = blk * 4 + i4
                        nc.tensor.matmul(
                            pss[i4],
                            ct_sb[:, kt, ic * 128 : (ic + 1) * 128],
                            qpw3t_sb[:, kt * Q_LEN : (kt + 1) * Q_LEN],
                            start=(kt == 0),
                            stop=(kt == KT - 1),
                        )
                for i4 in range(4):
                    ic = blk * 4 + i4
                    nc.scalar.activation(
                        out=e1_sb[:, ic, :], in_=pss[i4], func=EXP,
                        bias=u_sb[:, ic : ic + 1],
                    )
            for ic in range(8, IC):
                ps = ps_mm.tile([128, Q_LEN], F32, tag="mm")
                for kt in range(KT):
                    nc.tensor.matmul(
                        ps,
                        ct_sb[:, kt, ic * 128 : (ic + 1) * 128],
                        qpw3t_sb[:, kt * Q_LEN : (kt + 1) * Q_LEN],
                        start=(kt == 0),
                        stop=(kt == KT - 1),
                    )
                nc.scalar.activation(
                    out=e1_sb[:, ic, :], in_=ps, func=EXP, bias=u_sb[:, ic : ic + 1]
                )

            # ---- P6: T1 = E1^T @ C and cs1 = colsum(E1), then T = T1/cs1 ----
            # (before P5: it needs only E1, and P7 needs its T output)
            for jc in range(JC):
                t_ps = ps_mm.tile([128, DIM], F32, tag="mm")
                cs_ps = ps_vec.tile([128, 2], F32, tag="vec")
                for ic in range(IC):
                    lhsT = e1_sb[:, ic, jc * 128 : (jc + 1) * 128]
                    nc.tensor.matmul(
                        t_ps, lhsT, cn_sb[:, ic, :],
                        start=(ic == 0), stop=(ic == IC - 1),
                    )
                    nc.tensor.matmul(
                        cs_ps, lhsT, ones_r,
                        start=(ic == 0), stop=(ic == IC - 1),
                    )
                nc.vector.reciprocal(out=rcs_sb[:, jc : jc + 1], in_=cs_ps[:, 0:1])
                nc.scalar.mul(
                    t_sb[:, jc * DIM : (jc + 1) * DIM], t_ps, rcs_sb[:, jc : jc + 1]
                )

            # ---- P5+P7 fused by nn: P7's ic group only needs the E2 columns
            # in its own 512-wide nn slice, so outputs start ~20us earlier ----
            for nn in range(4):
                for jc in range(JC):
                    ps = ps_mm.tile([128, 512], F32, tag="mm")
                    for kt in range(KT):
                        nc.tensor.matmul(
                            ps,
                            qpw3t_sb[
                                :, kt * Q_LEN + jc * 128 : kt * Q_LEN + (jc + 1) * 128
                            ],
                            ct_sb[:, kt, nn * 512 : (nn + 1) * 512],
                            start=(kt == 0),
                            stop=(kt == KT - 1),
                        )
                    nc.scalar.activation(
                        out=e2_sb[:, jc, nn * 512 : (nn + 1) * 512],
                        in_=ps,
                        func=EXP,
                        bias=v_sb[:, jc : jc + 1],
                    )
                for ic in range(nn * 4, nn * 4 + 4):
                    a_ps = ps_mm.tile([128, DIM], F32, tag="mm")
                    b_ps = ps_mm.tile([128, DIM], F32, tag="mm")
                    r_ps = ps_vec.tile([128, 2], F32, tag="vec")
                    for jc in range(JC):
                        lhsT = e2_sb[:, jc, ic * 128 : (ic + 1) * 128]
                        nc.tensor.matmul(
                            a_ps, lhsT, qp_sb[:, jc, :],
                            start=(jc == 0), stop=(jc == JC - 1),
                        )
                        nc.tensor.matmul(
                            b_ps, lhsT, t_sb[:, jc * DIM : (jc + 1) * DIM],
                            start=(jc == 0), stop=(jc == JC - 1),
                        )
                        nc.tensor.matmul(
                            r_ps, lhsT, ones_r,
                            start=(jc == 0), stop=(jc == JC - 1),
                        )
                    rr = scr.tile([128, 1], F32, tag="rr")
                    nc.vector.reciprocal(out=rr, in_=r_ps[:, 0:1])
                    st = stagep.tile([128, 3 * DIM], F32, tag="stage")
                    bm = scr.tile([128, DIM], F32, tag="bm")
                    nc.scalar.mul(st[:, 0:DIM], a_ps, rr)
                    nc.vector.tensor_scalar_mul(bm, b_ps, rr)
                    if debug_taps and ic == 0:
                        nc.sync.dma_start(out=B_TAP[:], in_=bm)
                        nc.sync.dma_start(out=R_TAP[:], in_=rr)
                    cnf = cn_sb[:, ic, :].bitcast(F32)
                    nc.vector.tensor_mul(st[:, DIM : 2 * DIM], st[:, 0:DIM], cnf)
                    nc.sync.dma_start(
                        out=Y[ic * 128 : (ic + 1) * 128, DIM : 3 * DIM],
                        in_=st[:, 0 : 2 * DIM],
                    )
                    nc.vector.tensor_mul(st[:, 2 * DIM : 3 * DIM], bm, cnf)
                    nc.sync.dma_start(
                        out=Y[ic * 128 : (ic + 1) * 128, 3 * DIM : 4 * DIM],
                        in_=st[:, 2 * DIM : 3 * DIM],
                    )

            if debug_taps:
                nc.sync.dma_start(out=T_TAP[:], in_=t_sb.bitcast(F32))

    if split_waits:
        _split_multi_waits(nc)
    return nc


_PROGRAM = None


def _get_program():
    global _PROGRAM
    if _PROGRAM is None:
        _PROGRAM = _build_program()
    return _PROGRAM


def kernel(C, Q, C_mask, Q_mask, Wq, bq, w1, w2, w3, b):
    # Masks are all-ones and bq/b are zero for this problem (spec fills);
    # they cancel out of the computation and are not shipped to the device.
    C = np.asarray(C, np.float32)
    Q = np.asarray(Q, np.float32)
    Wq = np.ascontiguousarray(np.asarray(Wq, np.float32))
    w1 = np.asarray(w1, np.float32)
    w2 = np.asarray(w2, np.float32)
    w3 = np.asarray(w3, np.float32)

    w1b = np.ascontiguousarray(np.broadcast_to(w1[None, :], (128, DIM)))
    w2b = np.ascontiguousarray(np.broadcast_to(w2[None, :], (128, DIM)))
    w3c = np.ascontiguousarray(w3.reshape(KT, 128).T)
    onesc = np.zeros((128, 2), np.float32); onesc[:, 0] = 1.0

    in_maps = []
    for bi in range(B):
        Cb = np.ascontiguousarray(C[:, bi, :])
        in_maps.append(
            {
                "CT": np.ascontiguousarray(Cb.T),
                "CN": Cb,
                "QT": np.ascontiguousarray(Q[:, bi, :].T),
                "WQ": Wq,
                "W1B": w1b,
                "W2B": w2b,
                "W3C": w3c,
                "ONESC": onesc,
            }
        )

    nc = _get_program()
    res = run_bass_kernel_spmd(nc, in_maps, list(range(N_CORES)))
    return np.stack([res.results[c]["Y"] for c in range(N_CORES)], axis=1)



# revision 5
# speedup vs baseline: 1.1037x; 1.1037x over previous
"""Coattention kernel for Trainium2, data-parallel over batch across 8 NeuronCores.

v2: fp16 matmul operands (same 1 cyc/row PE rate as f32r, half the DMA bytes)
and the S0^T recompute is replaced by a PE transpose of E1:

    E1[i,j]  = exp(S0[i,j] + u_i)            (col-softmax numerator)
    E1T      = PE-transpose(E1)              [q, c]
    A_i      = (sum_j E1T[j,i] e^{v_j} Qp_j) / r_i,  r_i = sum_j E1T[j,i] e^{v_j}
    (the e^{u_i} factor in E1T cancels between numerator and r_i, so the
     row-softmax path needs no second exp; e^{v_j} is folded into the
     matmul RHS operands qv = e^v*Qp, tv = e^v*T/cs, evcol = e^v)
    T_j      = (sum_i E1[i,j] C_i) / cs_j,   cs_j = sum_i E1[i,j]
    Bm       = (E1T^T @ tv) / r
    out cols = [A | C*A | C*Bm]   (C passthrough cols are assembled on host)

u and v are tiny PE matmuls (rhs = w1 / Wq@w2 column); masks are all-ones
and b == bq == 0 per the problem spec, so they drop out.
"""

import os
import sys

import numpy as np

for _p in ("/opt/trn_rl_repo", "/root/.axon_site/_ro/trn_rl_repo"):
    if os.path.isdir(_p) and _p not in sys.path:
        sys.path.append(_p)

import concourse.bass as bass
import concourse.mybir as mybir
import concourse.tile as tile
from concourse.bass_utils import run_bass_kernel_spmd

C_LEN, Q_LEN, DIM, B = 2048, 512, 512, 8
N_CORES = 8
IC = C_LEN // 128   # 16 i-chunks
JC = Q_LEN // 128   # 4 j-chunks
KT = DIM // 128     # 4 k-tiles

F32 = mybir.dt.float32
F16 = mybir.dt.float16
EXP = mybir.ActivationFunctionType.Exp


def _split_multi_waits(nc, cap=1):
    """Walrus in this container rejects >1 sync wait per CTRL instruction;
    Tile's tail drain carries one wait per tracked processor.  Spill the
    extras onto preceding single-wait NoOps on the same engine."""
    for fn in nc.m.functions:
        for blk in fn.blocks:
            insts = list(blk.instructions)
            out, changed = [], False
            for inst in insts:
                si = inst.sync_info
                ow = si.on_wait if si is not None else None
                if ow is not None and len(ow) > cap:
                    waits = list(ow)
                    for w in waits[:-cap]:
                        nop = mybir.InstNoOp(
                            name=nc.get_next_instruction_name(), ins=[], outs=[]
                        )
                        nop.engine = inst.engine
                        nop.sync_info = mybir.SyncInfo(on_wait=[w], on_update=[])
                        out.append(nop)
                    si.on_wait = waits[-cap:]
                    changed = True
                out.append(inst)
            if changed:
                blk.instructions = out


def _build_program(split_waits=True):
    nc = bass.Bass()

    QT = nc.dram_tensor("QT", [DIM, Q_LEN], F16, kind="ExternalInput")
    WQ = nc.dram_tensor("WQ", [DIM, DIM], F16, kind="ExternalInput")
    CT = nc.dram_tensor("CT", [DIM, C_LEN], F16, kind="ExternalInput")
    CN = nc.dram_tensor("CN", [C_LEN, DIM], F16, kind="ExternalInput")
    W3C = nc.dram_tensor("W3C", [128, KT], F32, kind="ExternalInput")
    W2B = nc.dram_tensor("W2B", [128, DIM], F32, kind="ExternalInput")
    W1R = nc.dram_tensor("W1R", [128, KT], F16, kind="ExternalInput")
    ONESC = nc.dram_tensor("ONESC", [128, 2], F16, kind="ExternalInput")
    IDT = nc.dram_tensor("IDT", [128, 128], F16, kind="ExternalInput")
    Y = nc.dram_tensor("Y", [C_LEN, 3 * DIM], F32, kind="ExternalOutput")

    with tile.TileContext(nc) as tc:
        with (
            tc.tile_pool(name="consts", bufs=1) as consts,
            tc.tile_pool(name="big", bufs=1) as big,
            tc.tile_pool(name="ps_mm", bufs=4, space="PSUM") as ps_mm,
            tc.tile_pool(name="ps_t", bufs=1, space="PSUM") as ps_t,
            tc.tile_pool(name="ps_vec", bufs=2, space="PSUM") as ps_vec,
            tc.tile_pool(name="scr", bufs=2) as scr,
            tc.tile_pool(name="stage", bufs=4) as stagep,
        ):
            # ---- inputs (qt/wq first: P1 needs them) ----
            qt_sb = big.tile([128, KT, Q_LEN], F16)
            wq_sb = big.tile([128, KT, DIM], F16)
            for kt in range(KT):
                nc.sync.dma_start(
                    out=wq_sb[:, kt, :], in_=WQ[kt * 128 : (kt + 1) * 128, :]
                )
                nc.gpsimd.dma_start(
                    out=qt_sb[:, kt, :], in_=QT[kt * 128 : (kt + 1) * 128, :]
                )

            w3c = consts.tile([128, KT], F32)
            w2b = consts.tile([128, DIM], F32)
            w1r = consts.tile([128, KT], F16)
            ones_c = consts.tile([128, 2], F16)
            idt = consts.tile([128, 128], F16)
            nc.sync.dma_start(out=w3c, in_=W3C[:])
            nc.sync.dma_start(out=w1r, in_=W1R[:])
            nc.sync.dma_start(out=ones_c, in_=ONESC[:])
            nc.sync.dma_start(out=idt, in_=IDT[:])
            nc.sync.dma_start(out=w2b, in_=W2B[:])

            ct_sb = big.tile([128, KT, C_LEN], F16)
            ctw3_sb = big.tile([128, KT, C_LEN], F16)
            cn_sb = big.tile([128, IC, DIM], F16)

            for kt in range(KT):
                nc.sync.dma_start(
                    out=ct_sb[:, kt, :], in_=CT[kt * 128 : (kt + 1) * 128, :]
                )
                # w3-scaled copy for the S0 lhsT (w3 is a per-partition
                # scalar in this [d, c] layout)
                nc.vector.tensor_scalar_mul(
                    ctw3_sb[:, kt, :], ct_sb[:, kt, :], w3c[:, kt : kt + 1]
                )
            for g in range(4):
                nc.gpsimd.dma_start(
                    out=cn_sb[:, g * 4 : (g + 1) * 4, :],
                    in_=CN[g * 512 : (g + 1) * 512, :].rearrange(
                        "(ic p) e -> p ic e", p=128
                    ),
                )

            qpt_sb = big.tile([128, KT, Q_LEN], F16)   # Qp^T plain
            qv_sb = big.tile([128, JC, DIM], F16)      # e^v * Qp natural
            tv_sb = big.tile([128, JC, DIM], F16)      # e^v * T / cs
            evcol = consts.tile([128, JC, 2], F16)     # [e^v | 0] r-matmul rhs
            e1_sb = big.tile([128, IC, Q_LEN], F16)    # exp(S0 + u), [c, q]
            e1t_sb = big.tile([128, JC, C_LEN], F16)   # E1 transposed, [q, c]

            u_sb = consts.tile([128, IC], F32)
            v_sb = consts.tile([128, JC], F32)
            ev_sb = consts.tile([128, JC], F32)

            # ---- P1: Qp^T (per d-out chunk) ----
            for m in range(KT):
                ps = ps_mm.tile([128, Q_LEN], F32, tag="mm")
                for kt in range(KT):
                    nc.tensor.matmul(
                        ps,
                        wq_sb[:, kt, m * 128 : (m + 1) * 128],
                        qt_sb[:, kt, :],
                        start=(kt == 0),
                        stop=(kt == KT - 1),
                    )
                nc.scalar.copy(qpt_sb[:, m, :], ps)

            # ---- P2: Qp natural + v + e^v + qv ----
            for jc in range(JC):
                ps = ps_mm.tile([128, DIM], F32, tag="mm")
                for kt in range(KT):
                    nc.tensor.matmul(
                        ps,
                        qt_sb[:, kt, jc * 128 : (jc + 1) * 128],
                        wq_sb[:, kt, :],
                        start=(kt == 0),
                        stop=(kt == KT - 1),
                    )
                sc = scr.tile([128, DIM], F32, tag="ttr")
                nc.vector.tensor_mul(sc, ps, w2b)
                nc.vector.reduce_sum(
                    v_sb[:, jc : jc + 1], sc, axis=mybir.AxisListType.X
                )
                nc.scalar.activation(
                    out=ev_sb[:, jc : jc + 1], in_=v_sb[:, jc : jc + 1], func=EXP
                )
                nc.scalar.mul(qv_sb[:, jc, :], ps, ev_sb[:, jc : jc + 1])
            nc.vector.memset(evcol, 0.0)
            nc.vector.tensor_copy(evcol[:, :, 0], ev_sb)

            # ---- u via tiny PE matmuls: u = C @ w1 ----
            ps_u = ps_vec.tile([128, IC], F32, tag="vec")
            for ic in range(IC):
                for kt in range(KT):
                    nc.tensor.matmul(
                        ps_u[:, ic : ic + 1],
                        ct_sb[:, kt, ic * 128 : (ic + 1) * 128],
                        w1r[:, kt : kt + 1],
                        start=(kt == 0),
                        stop=(kt == KT - 1),
                    )
            nc.vector.tensor_copy(u_sb, ps_u)

            # ---- P4: S0 natural + E1 = exp(S0 + u) ----
            for blk in range(4):
                pss = []
                for _i in range(4):
                    ps_blk = ps_mm.tile([128, Q_LEN], F32, tag="mm")
                    pss.append(ps_blk)
                for kt in range(KT):
                    for i4 in range(4):
                        ic = blk * 4 + i4
                        nc.tensor.matmul(
                            pss[i4],
                            ctw3_sb[:, kt, ic * 128 : (ic + 1) * 128],
                            qpt_sb[:, kt, :],
                            start=(kt == 0),
                            stop=(kt == KT - 1),
                        )
                for i4 in range(4):
                    ic = blk * 4 + i4
                    nc.scalar.activation(
                        out=e1_sb[:, ic, :], in_=pss[i4], func=EXP,
                        bias=u_sb[:, ic : ic + 1],
                    )

            # ---- E1T via PE transpose + T/cs per jc ----
            for jc in range(JC):
                tp = ps_t.tile([128, C_LEN], F16, tag="tp")
                for ic in range(IC):
                    nc.tensor.matmul(
                        tp[:, ic * 128 : (ic + 1) * 128],
                        e1_sb[:, ic, jc * 128 : (jc + 1) * 128],
                        idt,
                        is_transpose=True,
                    )
                nc.vector.tensor_copy(e1t_sb[:, jc, :], tp)

                t_ps = ps_mm.tile([128, DIM], F32, tag="mm")
                cs_ps = ps_vec.tile([128, 2], F32, tag="vec")
                for ic in range(IC):
                    lhsT = e1_sb[:, ic, jc * 128 : (jc + 1) * 128]
                    nc.tensor.matmul(
                        t_ps, lhsT, cn_sb[:, ic, :],
                        start=(ic == 0), stop=(ic == IC - 1),
                    )
                    nc.tensor.matmul(
                        cs_ps, lhsT, ones_c,
                        start=(ic == 0), stop=(ic == IC - 1),
                    )
                rcs = scr.tile([128, 1], F32, tag="rr")
                nc.vector.reciprocal(out=rcs, in_=cs_ps[:, 0:1])
                tsc = scr.tile([128, 1], F32, tag="rr")
                nc.vector.tensor_mul(tsc, rcs, ev_sb[:, jc : jc + 1])
                nc.scalar.mul(tv_sb[:, jc, :], t_ps, tsc)

            # ---- P7: A, Bm, r per ic; stage and stream out ----
            for ic in range(IC):
                a_ps = ps_mm.tile([128, DIM], F32, tag="mm")
                b_ps = ps_mm.tile([128, DIM], F32, tag="mm")
                r_ps = ps_vec.tile([128, 2], F32, tag="vec")
                for jc in range(JC):
                    lhsT = e1t_sb[:, jc, ic * 128 : (ic + 1) * 128]
                    nc.tensor.matmul(
                        a_ps, lhsT, qv_sb[:, jc, :],
                        start=(jc == 0), stop=(jc == JC - 1),
                    )
                    nc.tensor.matmul(
                        b_ps, lhsT, tv_sb[:, jc, :],
                        start=(jc == 0), stop=(jc == JC - 1),
                    )
                    nc.tensor.matmul(
                        r_ps, lhsT, evcol[:, jc, :],
                        start=(jc == 0), stop=(jc == JC - 1),
                    )
                rr = scr.tile([128, 1], F32, tag="rr")
                nc.vector.reciprocal(out=rr, in_=r_ps[:, 0:1])
                st = stagep.tile([128, 3 * DIM], F32, tag="stage")
                bm = scr.tile([128, DIM], F32, tag="bm")
                nc.scalar.mul(st[:, 0:DIM], a_ps, rr)
                nc.vector.tensor_scalar_mul(bm, b_ps, rr)
                nc.vector.tensor_mul(st[:, DIM : 2 * DIM], st[:, 0:DIM], cn_sb[:, ic, :])
                nc.vector.tensor_mul(st[:, 2 * DIM : 3 * DIM], bm, cn_sb[:, ic, :])
                nc.sync.dma_start(
                    out=Y[ic * 128 : (ic + 1) * 128, :], in_=st
                )

    if split_waits:
        _split_multi_waits(nc)
    return nc


_PROGRAM = None


def _get_program():
    global _PROGRAM
    if _PROGRAM is None:
        _PROGRAM = _build_program()
    return _PROGRAM


def kernel(C, Q, C_mask, Q_mask, Wq, bq, w1, w2, w3, b):
    # Masks are all-ones and bq/b are zero for this problem (spec fills);
    # they cancel out of the computation and are not shipped to the device.
    C = np.asarray(C, np.float32)
    Q = np.asarray(Q, np.float32)
    Wq = np.asarray(Wq, np.float32)
    w1 = np.asarray(w1, np.float32)
    w2 = np.asarray(w2, np.float32)
    w3 = np.asarray(w3, np.float32)

    w3c = np.ascontiguousarray(w3.reshape(KT, 128).T)                  # f32
    w2b = np.ascontiguousarray(
        np.broadcast_to(w2[None, :], (128, DIM))
    ).astype(np.float32)
    w1r = np.ascontiguousarray(w1.reshape(KT, 128).T).astype(np.float16)
    onesc = np.zeros((128, 2), np.float16)
    onesc[:, 0] = 1.0
    idt = np.eye(128, dtype=np.float16)

    in_maps = []
    for bi in range(B):
        Cb = np.ascontiguousarray(C[:, bi, :])
        in_maps.append(
            {
                "QT": np.ascontiguousarray(Q[:, bi, :].T).astype(np.float16),
                "WQ": Wq.astype(np.float16),
                "CT": np.ascontiguousarray(Cb.T).astype(np.float16),
                "CN": Cb.astype(np.float16),
                "W3C": w3c,
                "W2B": w2b,
                "W1R": w1r,
                "ONESC": onesc,
                "IDT": idt,
            }
        )

    nc = _get_program()
    res = run_bass_kernel_spmd(nc, in_maps, list(range(N_CORES)))
    out = np.empty((C_LEN, B, 4 * DIM), np.float32)
    out[:, :, 0:DIM] = C  # passthrough columns assembled on host
    for c in range(N_CORES):
        out[:, c, DIM:] = res.results[c]["Y"]
    return out
